# revision 3
# baseline (speedup 1.0000x reference)
"""Grouped linear (MoE grouped GEMM) on 8 TRN2 NeuronCores via Bass/Tile.

Reference: out = ragged_dot(x, weight.swapaxes(1,2), group_lens) with
x [32768, 1024] fp32, weight [16, 1024, 1024] fp32, tokens pre-sorted
into 16 contiguous groups.

Strategy — token-parallel SPMD with host-side dispatch:
  * The host cuts each group's contiguous token run into "chunks" (one
    weight load each), each chunk into <=512-token sub-slots; an LPT
    packer balances chunks across the 8 cores.  All cores run ONE
    program whose shape is the per-position maximum profile; per-core
    numpy inputs decide which expert/tokens each position processes.
  * On-chip per sub-slot of width u: 8 out-blocks x 8 k-steps of
    [128x128] @ [128xu] bf16 matmuls accumulated in fp32 PSUM, PSUM ->
    SBUF copy (bf16), contiguous DMAs for all streams.
  * Inputs are pre-transposed/padded on the host so every DMA is
    contiguous per partition row; outputs are upcast & scattered back
    on the host.

Measured on trn2 (8 cores, seed-0 data): ~131 us/exec, rel err 3.7e-3
(bf16 compute + bf16 output quantization; fp32 accumulate).
"""

import numpy as np
import ml_dtypes

import concourse.bass as bass
import concourse.tile as tile
from concourse import bacc, mybir
from concourse.bass_utils import run_bass_kernel_spmd

G, NTOK, DIN, DOUT = 16, 32768, 1024, 1024
NCORES = 8
TT = 512           # max tokens per sub-slot
KT = DIN // 128    # 8 contraction sub-tiles
OB = DOUT // 128   # 8 output blocks
WALIGN = 16        # sub-slot width alignment (tokens)

_NC_CACHE: dict = {}


# ---------------------------------------------------------------- planner

def _split_even(total, maxpiece):
    np_ = -(-total // maxpiece)
    base = total // np_
    rem = total - base * np_
    return [base + (1 if i < rem else 0) for i in range(np_)]


def _mk_chunk(g, start, clen):
    widths = _split_even(clen, TT)
    offs = np.cumsum([0] + widths[:-1])
    return (g, [(int(start + o), int(n)) for o, n in zip(offs, widths)])


def _chunk_tok(ch):
    return sum(n for _, n in ch[1])


def _assemble(chunk_list):
    """LPT + chunk-count equalization + sorted alignment -> (profile, assign)."""
    chunks = sorted(chunk_list, key=lambda ch: -_chunk_tok(ch))
    loads = [0.0] * NCORES
    percore: list = [[] for _ in range(NCORES)]
    for ch in chunks:
        cost = sum(-(-n // WALIGN) * WALIGN for _, n in ch[1])
        c = min(range(NCORES), key=lambda i: (loads[i], len(percore[i])))
        loads[c] += cost
        percore[c].append(ch)
    maxn = max(len(pc) for pc in percore)
    for c in range(NCORES):
        while len(percore[c]) < maxn:
            percore[c].sort(key=lambda ch: -_chunk_tok(ch))
            big = percore[c][0]
            tok = _chunk_tok(big)
            if tok < 2 * WALIGN:
                break
            g = big[0]
            start = big[1][0][0]
            h1 = tok // 2
            percore[c] = [_mk_chunk(g, start, h1),
                          _mk_chunk(g, start + h1, tok - h1)] + percore[c][1:]
    for c in range(NCORES):
        percore[c].sort(key=lambda ch: (-len(ch[1]), -_chunk_tok(ch)))
    P = max(len(percore[c]) for c in range(NCORES))
    profile = []
    for p in range(P):
        m = max(len(percore[c][p][1]) if p < len(percore[c]) else 0
                for c in range(NCORES))
        widths = []
        for j in range(m):
            u = max(
                percore[c][p][1][j][1]
                if p < len(percore[c]) and j < len(percore[c][p][1])
                else 0
                for c in range(NCORES)
            )
            widths.append(-(-u // WALIGN) * WALIGN)
        profile.append(widths)
    assign = [
        [percore[c][p] if p < len(percore[c]) else None for p in range(P)]
        for c in range(NCORES)
    ]
    return profile, assign


def _plan_cost(profile):
    toks = sum(sum(w) for w in profile)
    pe_us = toks / TT * 13.6 + 6.0                    # MM stream + ramp/tail
    wmb = len(profile) * KT * DOUT * 128 * 2 / 1e6    # bf16 weights
    xmb = toks * KT * 128 * 2 / 1e6                   # bf16 activations
    omb = toks * OB * 128 * 2 / 1e6                   # bf16 outputs
    dma_us = (wmb + xmb + omb) / 0.35                 # ~350 GB/s effective
    return max(pe_us, dma_us * 1.05)


def _chunks_at_cap(group_lens, cap):
    edges = np.concatenate([[0], np.cumsum(np.asarray(group_lens, np.int64))])
    chunk_list = []
    for g in range(G):
        s, e = int(edges[g]), int(edges[g + 1])
        for clen in _split_even(e - s, cap) if e > s else []:
            chunk_list.append(_mk_chunk(g, s, clen))
            s += clen
    return chunk_list


def _plan(group_lens):
    best = None
    for cap in (4096, 3072, 2560, 2048, 1792, 1536, 1280, 1024,
                896, 768, 640, 512, 448, 384):
        profile, assign = _assemble(_chunks_at_cap(group_lens, cap))
        cost = _plan_cost(profile)
        if best is None or cost < best[0]:
            best = (cost, profile, assign)
    return best[1], best[2]


def _offsets(profile):
    xoff, ooff = [], []
    xl = ol = 0
    for widths in profile:
        xo, oo = [], []
        for u in widths:
            xo.append(xl)
            oo.append(ol)
            xl += KT * u
            ol += OB * u
        xoff.append(xo)
        ooff.append(oo)
    return xoff, ooff, xl, ol


# ------------------------------------------------------------- bass build

def _build(profile, reps=1):
    key = (tuple(tuple(w) for w in profile), reps)
    if key in _NC_CACHE:
        return _NC_CACHE[key]
    dt_in = mybir.dt.bfloat16
    dt_out = mybir.dt.bfloat16
    xoff, ooff, XL, OL = _offsets(profile)
    P = len(profile)

    nc = bacc.Bacc(None, target_bir_lowering=False)
    xt = nc.declare_dram_parameter("xt", [128, XL], dt_in, isOutput=False)
    wt = nc.declare_dram_parameter("wt", [128, P * KT * DOUT], dt_in, isOutput=False)
    ot = nc.declare_dram_parameter("ot", [128, OL], dt_out, isOutput=True)

    with tile.TileContext(nc) as tc:
        with (
            tc.tile_pool(name="wp", bufs=3) as wpool,
            tc.tile_pool(name="xp", bufs=3) as xpool,
            tc.tile_pool(name="op", bufs=3) as opool,
            tc.tile_pool(name="ps", bufs=8, space=bass.MemorySpace.PSUM) as pspool,
        ):
          for _rep in range(reps):
            for p, widths in enumerate(profile):
                wsb = wpool.tile([128, KT * DOUT], dt_in, tag="wsb")
                if p == 0:
                    # split the first weight DMA per k-step so PE starts early
                    for k in range(KT):
                        nc.sync.dma_start(
                            wsb[:, k * DOUT : (k + 1) * DOUT],
                            wt[:, k * DOUT : (k + 1) * DOUT],
                        )
                else:
                    nc.sync.dma_start(
                        wsb[:, :], wt[:, p * KT * DOUT : (p + 1) * KT * DOUT]
                    )
                for j, u in enumerate(widths):
                    xsb = xpool.tile([128, KT * TT], dt_in, tag="xsb")
                    osb = opool.tile([128, OB * TT], dt_out, tag="osb")
                    if p == 0 and j == 0:
                        for k in range(KT):
                            nc.sync.dma_start(
                                xsb[:, k * u : (k + 1) * u],
                                xt[:, xoff[p][j] + k * u : xoff[p][j] + (k + 1) * u],
                            )
                    else:
                        nc.sync.dma_start(
                            xsb[:, : KT * u], xt[:, xoff[p][j] : xoff[p][j] + KT * u]
                        )
                    for o in range(OB):
                        ps = pspool.tile([128, TT], mybir.dt.float32, tag="ps")
                        for k in range(KT):
                            nc.tensor.matmul(
                                ps[:, :u],
                                wsb[:, k * DOUT + o * 128 : k * DOUT + (o + 1) * 128],
                                xsb[:, k * u : (k + 1) * u],
                                start=(k == 0),
                                stop=(k == KT - 1),
                            )
                        nc.vector.tensor_copy(osb[:, o * u : (o + 1) * u], ps[:, :u])
                    last = p == len(profile) - 1 and j == len(widths) - 1
                    if last:
                        # split the final out DMA per o-block to drain early
                        for o in range(OB):
                            nc.sync.dma_start(
                                ot[:, ooff[p][j] + o * u : ooff[p][j] + (o + 1) * u],
                                osb[:, o * u : (o + 1) * u],
                            )
                    else:
                        nc.sync.dma_start(
                            ot[:, ooff[p][j] : ooff[p][j] + OB * u], osb[:, : OB * u]
                        )

    nc.compile()
    _NC_CACHE[key] = nc
    return nc


# ----------------------------------------------------------- host scatter

def _prep_inputs(x, weight, profile, assign):
    xoff, ooff, XL, OL = _offsets(profile)
    P = len(profile)
    xbf = x.astype(ml_dtypes.bfloat16)
    # wpm[g][p, k*DOUT + o] = weight[g, o, k*128+p]
    wpm = np.ascontiguousarray(
        weight.reshape(G, DOUT, KT, 128).transpose(0, 3, 2, 1)
    ).astype(ml_dtypes.bfloat16).reshape(G, 128, KT * DOUT)
    in_maps = []
    for c in range(NCORES):
        xtc = np.zeros((128, XL), ml_dtypes.bfloat16)
        wtc = np.zeros((128, P * KT * DOUT), ml_dtypes.bfloat16)
        for p, widths in enumerate(profile):
            ch = assign[c][p]
            if ch is None:
                continue
            g, tlist = ch
            wtc[:, p * KT * DOUT : (p + 1) * KT * DOUT] = wpm[g]
            for j, (s, n) in enumerate(tlist):
                u = widths[j]
                b = np.zeros((u, DIN), ml_dtypes.bfloat16)
                b[:n] = xbf[s : s + n]
                xtc[:, xoff[p][j] : xoff[p][j] + KT * u] = (
                    b.reshape(u, KT, 128).transpose(2, 1, 0).reshape(128, KT * u)
                )
        in_maps.append({"xt": xtc, "wt": wtc})
    return in_maps


def _gather_out(results, profile, assign):
    xoff, ooff, XL, OL = _offsets(profile)
    out = np.empty((NTOK, DOUT), np.float32)
    for c in range(NCORES):
        otc = np.asarray(results[c]["ot"]).astype(np.float32)
        for p, widths in enumerate(profile):
            ch = assign[c][p]
            if ch is None:
                continue
            _, tlist = ch
            for j, (s, n) in enumerate(tlist):
                u = widths[j]
                blk = otc[:, ooff[p][j] : ooff[p][j] + OB * u].reshape(128, OB, u)
                out[s : s + n] = blk.transpose(2, 1, 0).reshape(u, DOUT)[:n]
    return out


def kernel(x, weight, group_lens):
    x = np.ascontiguousarray(np.asarray(x))
    weight = np.ascontiguousarray(np.asarray(weight))
    profile, assign = _plan(group_lens)
    nc = _build(profile)
    in_maps = _prep_inputs(x, weight, profile, assign)
    res = run_bass_kernel_spmd(nc, in_maps, list(range(NCORES)))
    return _gather_out(res.results, profile, assign)

